# revision 1
# baseline (speedup 1.0000x reference)
"""Trainium2 Bass kernel for CIN: out[b,m,d] = sigmoid(einsum('bid,bjd,ijm', x0, x, K)).

Shapes (hardcoded): x0,x [4096, 40, 64] f32, kernel [40, 40, 128] f32,
out [4096, 128, 64] f32.

Sharding: data-parallel over batch B across 8 NeuronCores (512 b each);
kernel tensor replicated (prepacked on host into matmul chunk layout).

Per-core pipeline (groups of 8 b's; free dim = 8*64 = 512):
  - DMA x0/x group slices as [40, (b d)] tiles (partition = i)
  - PE-transpose 128-col blocks -> [(b d), i] tiles
  - DVE outer product with stride-0 broadcast APs:
        Z3[(b d), (i j)] = x0b[., i] * xb[., j]          (the CIN interaction)
  - PE-transpose Z3 128-chunks -> Z[(i j)-chunk, (b d)]  (matmul rhs layout)
  - 13 PSUM-accumulated matmuls vs prepacked kernel chunks (contraction i j)
  - sigmoid fused into PSUM evacuation on ACT, DMA out
"""

import sys

for _p in ("/opt/trn_rl_repo", "/root/.axon_site/_ro/trn_rl_repo"):
    if _p not in sys.path:
        sys.path.insert(0, _p)

from contextlib import ExitStack

import numpy as np

import concourse.bass as bass
from concourse import bacc
import concourse.tile as tile
from concourse import mybir
from concourse.bass_utils import run_bass_kernel_spmd
from concourse.masks import make_identity

B, F0, F, D, M = 4096, 40, 40, 64, 128
NCORES = 8
NB = B // NCORES            # 512 b per core
IJ = F0 * F                 # 1600
NCHUNK = (IJ + 127) // 128  # 13
IJPAD = NCHUNK * 128        # 1664

f32 = mybir.dt.float32


def _pack_kernel(kernel_np: np.ndarray) -> np.ndarray:
    kf = kernel_np.reshape(IJ, M).astype(np.float32)
    kp = np.zeros((IJPAD, M), dtype=np.float32)
    kp[:IJ] = kf
    return np.ascontiguousarray(kp.reshape(NCHUNK, 128, M))


def _build(nb: int, gb: int = 8, evac_split: int = 2):
    ng = nb // gb
    free = gb * D
    ntiles = free // 128
    z_dtype = f32

    nc = bacc.Bacc("TRN2", num_devices=8)
    x0s = nc.declare_dram_parameter("x0s", [nb, F0, D], f32, isOutput=False)
    xs = nc.declare_dram_parameter("xs", [nb, F, D], f32, isOutput=False)
    kp = nc.declare_dram_parameter("kp", [NCHUNK, 128, M], z_dtype, isOutput=False)
    outp = nc.declare_dram_parameter("out", [nb, M, D], f32, isOutput=True)

    with ExitStack() as ctx:
        tc = ctx.enter_context(tile.TileContext(nc))
        singles = ctx.enter_context(tc.tile_pool(name="singles", bufs=1))
        xt_pool = ctx.enter_context(tc.tile_pool(name="xt", bufs=4))
        xb_pool = ctx.enter_context(tc.tile_pool(name="xb", bufs=3))
        z3_pool = ctx.enter_context(tc.tile_pool(name="z3", bufs=3))
        zc_pool = ctx.enter_context(tc.tile_pool(name="zc", bufs=2))
        osb_pool = ctx.enter_context(tc.tile_pool(name="osb", bufs=3))
        psx_pool = ctx.enter_context(tc.tile_pool(name="psx", bufs=4, space="PSUM"))
        pso_pool = ctx.enter_context(tc.tile_pool(name="pso", bufs=2, space="PSUM"))

        kw = singles.tile([128, NCHUNK, M], z_dtype)
        nc.sync.dma_start(out=kw, in_=kp.transpose([1, 0, 2]))

        id40 = singles.tile([F0, F0], f32)
        make_identity(nc, id40)
        id128 = singles.tile([128, 128], z_dtype)
        make_identity(nc, id128)
        # Dummy transpose: makes PE's clock aware of the GPSIMD identity
        # writes up front. The S3 LDWEIGHTS slot fits only ONE sync wait, so
        # real transposes must not need both a Pool wait and a DMA wait.
        ps_dummy = psx_pool.tile([128, 128], z_dtype, tag="psx")
        nc.tensor.transpose(ps_dummy, id128, id128)

        for g in range(ng):
            bsl = slice(g * gb, (g + 1) * gb)
            x0T = xt_pool.tile([F0, gb, D], f32, tag="x0T")
            xT = xt_pool.tile([F, gb, D], f32, tag="xT")
            nc.sync.dma_start(out=x0T, in_=x0s[bsl].transpose([1, 0, 2]))
            nc.sync.dma_start(out=xT, in_=xs[bsl].transpose([1, 0, 2]))
            x0Tf = x0T.rearrange("i b d -> i (b d)")
            xTf = xT.rearrange("i b d -> i (b d)")

            x0b = xb_pool.tile([128, ntiles, F0], f32, tag="x0b")
            xb = xb_pool.tile([128, ntiles, F], f32, tag="xb")
            for t in range(ntiles):
                csl = slice(t * 128, (t + 1) * 128)
                for src, dst in ((x0Tf, x0b), (xTf, xb)):
                    ps = psx_pool.tile([128, 128], f32, tag="psx")
                    nc.tensor.transpose(ps[:, :F0], src[:, csl], id40)
                    nc.vector.tensor_copy(dst[:, t, :], ps[:, :F0])

            zc = zc_pool.tile([128, NCHUNK, free], z_dtype, tag="zc")
            for t in range(ntiles):
                z3 = z3_pool.tile([128, IJPAD], z_dtype, tag="z3")
                if IJPAD > IJ:
                    nc.vector.memset(z3[:, IJ:IJPAD], 0.0)
                zv = z3[:, 0:IJ].rearrange("p (i j) -> p i j", i=F0)
                in0 = x0b[:, t, :].unsqueeze(2).broadcast_to((128, F0, F))
                in1 = xb[:, t, :].unsqueeze(1).broadcast_to((128, F0, F))
                nc.vector.tensor_tensor(out=zv, in0=in0, in1=in1,
                                        op=mybir.AluOpType.mult)
                for c in range(NCHUNK):
                    pst = psx_pool.tile([128, 128], z_dtype, tag="psx")
                    nc.tensor.transpose(pst, z3[:, c * 128:(c + 1) * 128], id128)
                    dst = zc[:, c, t * 128:(t + 1) * 128]
                    if (c + t) % evac_split == 0:
                        nc.scalar.copy(dst, pst)
                    else:
                        nc.vector.tensor_copy(dst, pst)

            pso = pso_pool.tile([128, free], f32, tag="pso")
            for c in range(NCHUNK):
                nc.tensor.matmul(pso, kw[:, c, :], zc[:, c, :],
                                 start=(c == 0), stop=(c == NCHUNK - 1))

            osb = osb_pool.tile([128, gb, D], f32, tag="osb")
            nc.scalar.activation(osb.rearrange("m b d -> m (b d)"), pso,
                                 mybir.ActivationFunctionType.Sigmoid)
            nc.sync.dma_start(out=outp[bsl].transpose([1, 0, 2]), in_=osb)

    nc.finalize()
    return nc


_NC_CACHE = {}


def _get_nc():
    if "nc" not in _NC_CACHE:
        _NC_CACHE["nc"] = _build(NB)
    return _NC_CACHE["nc"]


def kernel(x0: np.ndarray, x: np.ndarray, kernel: np.ndarray) -> np.ndarray:
    x0 = np.ascontiguousarray(np.asarray(x0, dtype=np.float32))
    x = np.ascontiguousarray(np.asarray(x, dtype=np.float32))
    kpacked = _pack_kernel(np.asarray(kernel, dtype=np.float32))

    nc = _get_nc()
    core_ids = list(range(NCORES))
    in_maps = [
        {
            "x0s": x0[i * NB:(i + 1) * NB],
            "xs": x[i * NB:(i + 1) * NB],
            "kp": kpacked,
        }
        for i in core_ids
    ]
    res = run_bass_kernel_spmd(nc, in_maps, core_ids)
    out = np.concatenate([np.asarray(r["out"]) for r in res.results], axis=0)
    return out.astype(np.float32)



# revision 3
# speedup vs baseline: 3.2966x; 3.2966x over previous
"""Trainium2 Bass kernel for CIN: out[b,m,d] = sigmoid(einsum('bid,bjd,ijm', x0, x, K)).

Shapes (hardcoded): x0,x [4096, 40, 64] f32, kernel [40, 40, 128] f32,
out [4096, 128, 64] f32.

Sharding: data-parallel over batch B across 8 NeuronCores (512 b each);
kernel tensor replicated (prepacked on host into chunked contraction
layout).

Transpose-free per-core pipeline (groups of 8 b's; free dim = 8*64 = 512):
  (i,j)-contraction space (1600) is packed into 14 chunks of 3 i's x 40
  j's = 120 partition rows. Per chunk c:
    - PE "replication matmul": psa = pp_c.T @ x0t, where pp_c is a 0/1
      [40, 128] matrix -> psa[o, bd] = x0[3c + o//40, bd]  (partition
      broadcast of x0 rows, done on the PE instead of impossible
      stride-0 partition APs)
    - DVE: A_c = psa * xr  (xr = x rows tiled 3x along partitions,
      loaded by plain DMA) -> A_c[p, bd] = x0[i(p),bd] * x[j(p),bd]
    - PE: pso += kc_c.T @ A_c  (kc_c[p, m] = K[i(p), j(p), m])
  After 14 chunks: ACT applies sigmoid during PSUM evacuation, DMA out.

All matmul operands are bf16 (fp32 PSUM accumulation); inputs are cast
to bf16 on the host, outputs are f32.
"""

import sys

for _p in ("/opt/trn_rl_repo", "/root/.axon_site/_ro/trn_rl_repo"):
    if _p not in sys.path:
        sys.path.insert(0, _p)

from contextlib import ExitStack

import numpy as np
import ml_dtypes

import concourse.bass as bass
from concourse import bacc
import concourse.tile as tile
from concourse import mybir
from concourse.bass_utils import run_bass_kernel_spmd

B, F0, F, D, M = 4096, 40, 40, 64, 128
NCORES = 8
NB = B // NCORES            # 512 b per core
GB = 8                      # b's per group
FREE = GB * D               # 512 free columns per matmul
NG = NB // GB               # 64 groups per core
NCH = 14                    # ceil(40/3) chunks of 3 i's x 40 j's
CONTRACT = 120              # useful contraction rows per chunk

f32 = mybir.dt.float32
bf16 = mybir.dt.bfloat16
BF16NP = ml_dtypes.bfloat16


def _pack_consts(kernel_np: np.ndarray):
    """kc[p, c, m] = K[3c + p//40, p%40, m]; pp[i, c, o] = [i == 3c + o//40]."""
    kc = np.zeros((128, NCH, M), np.float32)
    pp = np.zeros((F0, NCH, 128), np.float32)
    for c in range(NCH):
        for t in range(3):
            i = 3 * c + t
            if i < F0:
                kc[t * 40:(t + 1) * 40, c, :] = kernel_np[i]
                pp[i, c, t * 40:(t + 1) * 40] = 1.0
    return kc.astype(BF16NP), pp.astype(BF16NP)


def _build():
    nc = bacc.Bacc("TRN2", num_devices=NCORES)
    x0h = nc.declare_dram_parameter("x0h", [F0, NG, FREE], bf16, isOutput=False)
    xh = nc.declare_dram_parameter("xh", [F, NG, FREE], bf16, isOutput=False)
    kch = nc.declare_dram_parameter("kch", [128, NCH, M], bf16, isOutput=False)
    pph = nc.declare_dram_parameter("pph", [F0, NCH, 128], bf16, isOutput=False)
    outp = nc.declare_dram_parameter("out", [NB, M, D], f32, isOutput=True)

    with ExitStack() as ctx:
        tc = ctx.enter_context(tile.TileContext(nc))
        singles = ctx.enter_context(tc.tile_pool(name="singles", bufs=1))
        xr_pool = ctx.enter_context(tc.tile_pool(name="xr", bufs=3))
        x0_pool = ctx.enter_context(tc.tile_pool(name="x0", bufs=3))
        a_pool = ctx.enter_context(tc.tile_pool(name="a", bufs=4))
        osb_pool = ctx.enter_context(tc.tile_pool(name="osb", bufs=3))
        psa_pool = ctx.enter_context(tc.tile_pool(name="psa", bufs=4, space="PSUM"))
        pso_pool = ctx.enter_context(tc.tile_pool(name="pso", bufs=2, space="PSUM"))

        kcw = singles.tile([128, NCH, M], bf16)
        nc.sync.dma_start(out=kcw, in_=kch[:, :, :])
        ppw = singles.tile([F0, NCH, 128], bf16)
        nc.sync.dma_start(out=ppw, in_=pph[:, :, :])

        LOOKAHEAD = 2
        for g in range(NG):
            bsl = slice(g * GB, (g + 1) * GB)
            xr = xr_pool.tile([CONTRACT, FREE], bf16, tag="xr")
            for r in range(3):
                nc.sync.dma_start(out=xr[r * 40:(r + 1) * 40, :], in_=xh[:, g, :])
            x0t = x0_pool.tile([F0, FREE], bf16, tag="x0t")
            nc.sync.dma_start(out=x0t, in_=x0h[:, g, :])

            pso = pso_pool.tile([M, FREE], f32, tag="pso")
            acs = [None] * NCH
            # Software-pipelined issue order: replication matmul for chunk
            # c+LOOKAHEAD is enqueued on the PE before the contraction
            # matmul of chunk c, so the PE never sits behind the DVE
            # multiply in its own in-order queue.
            for c in range(NCH + LOOKAHEAD):
                if c < NCH:
                    psa = psa_pool.tile([128, FREE], f32, tag="psa")
                    nc.tensor.matmul(psa, ppw[:, c, :], x0t, start=True, stop=True)
                    ac = a_pool.tile([CONTRACT, FREE], bf16, tag="ac")
                    nc.vector.tensor_tensor(out=ac, in0=psa[0:CONTRACT, :], in1=xr,
                                            op=mybir.AluOpType.mult)
                    acs[c] = ac
                if c >= LOOKAHEAD:
                    cc = c - LOOKAHEAD
                    nc.tensor.matmul(pso, kcw[0:CONTRACT, cc, :], acs[cc],
                                     start=(cc == 0), stop=(cc == NCH - 1))

            osb = osb_pool.tile([M, GB, D], f32, tag="osb")
            nc.scalar.activation(osb.rearrange("m b d -> m (b d)"), pso,
                                 mybir.ActivationFunctionType.Sigmoid)
            nc.sync.dma_start(out=outp[bsl].transpose([1, 0, 2]), in_=osb)

    nc.finalize()
    return nc


_NC_CACHE = {}


def _get_nc():
    if "nc" not in _NC_CACHE:
        _NC_CACHE["nc"] = _build()
    return _NC_CACHE["nc"]


def _in_maps(x0: np.ndarray, x: np.ndarray, kernel_np: np.ndarray):
    kc, pp = _pack_consts(np.asarray(kernel_np, dtype=np.float32))
    x0 = np.asarray(x0, dtype=np.float32).astype(BF16NP)
    x = np.asarray(x, dtype=np.float32).astype(BF16NP)
    maps = []
    for i in range(NCORES):
        sl = slice(i * NB, (i + 1) * NB)
        x0c = np.ascontiguousarray(
            x0[sl].transpose(1, 0, 2).reshape(F0, NG, FREE))
        xc = np.ascontiguousarray(
            x[sl].transpose(1, 0, 2).reshape(F, NG, FREE))
        maps.append({"x0h": x0c, "xh": xc, "kch": kc, "pph": pp})
    return maps


def kernel(x0: np.ndarray, x: np.ndarray, kernel: np.ndarray) -> np.ndarray:
    nc = _get_nc()
    in_maps = _in_maps(x0, x, kernel)
    res = run_bass_kernel_spmd(nc, in_maps, list(range(NCORES)))
    out = np.concatenate([np.asarray(r["out"]) for r in res.results], axis=0)
    return out.astype(np.float32)


# revision 5
# speedup vs baseline: 5.1093x; 1.5499x over previous
"""Trainium2 Bass kernel for CIN: out[b,m,d] = sigmoid(einsum('bid,bjd,ijm', x0, x, K)).

v5 design notes (from microbenchmarks):
  - PE matmuls pipeline at ~216 ns (N=512, warm) ONLY when consecutive
    matmuls keep the SAME contraction row-config; mixing K=40/K=120
    serializes to ~540 ns. So every matmul here is K=120.
  - The x0 partition-broadcast must go through PE (0/1 replication
    matmul) + PSUM; PSUM-sourced DVE ops run at 1x (690 ns/chunk).
    PACKED mode halves both: the replication matmul runs in fp32 over
    host-packed bf16 PAIRS (fp32 x 1.0 passes bits through exactly), so
    it pumps only 256 columns, and the DVE reads PSUM via a bf16
    bitcast at 2x (333 ns/chunk).
  - Optionally a few chunks are routed ACT-copy + GPSIMD-multiply to
    offload the DVE.

Per-group pipeline (8 b's, free=512):
  repl MM (fp32 packed, K=120) -> psa; DVE/GP: ac = bitcast(psa)*xr;
  14 bf16 K=120 contraction MMs accumulate -> PSUM; ACT sigmoid; DMA.
"""

import sys

for _p in ("/opt/trn_rl_repo", "/root/.axon_site/_ro/trn_rl_repo"):
    if _p not in sys.path:
        sys.path.insert(0, _p)

from contextlib import ExitStack

import numpy as np
import ml_dtypes

import concourse.bass as bass
from concourse import bacc
import concourse.tile as tile
from concourse import mybir
from concourse.bass_utils import run_bass_kernel_spmd

B, F0, F, D, M = 4096, 40, 40, 64, 128
NCORES = 8
NB = B // NCORES            # 512
GB = 8
FREE = GB * D               # 512
NG = NB // GB               # 64
NCH = 14
CONTRACT = 120

f32 = mybir.dt.float32
bf16 = mybir.dt.bfloat16
BF16NP = ml_dtypes.bfloat16

PACKED = True               # fp32 packed-pair replication + bitcast evac
N_GP = 3                    # chunks routed ACT-copy + GPSIMD multiply
LOOKAHEAD = 2


def _pack_consts(kernel_np: np.ndarray):
    """kc[p, c, m] = K[3c+p//40, p%40, m]; pp[p, c, o] = [p == 3c + o//40]
    (pp rows 40-119 are zero; x0t rows 40-119 are finite copies)."""
    kc = np.zeros((CONTRACT, NCH, M), np.float32)
    pp = np.zeros((CONTRACT, NCH, 128), np.float32)
    for c in range(NCH):
        for t in range(3):
            i = 3 * c + t
            if i < F0:
                kc[t * 40:(t + 1) * 40, c, :] = kernel_np[i]
                pp[i, c, t * 40:(t + 1) * 40] = 1.0
    return kc.astype(BF16NP), pp.astype(np.float32 if PACKED else BF16NP)


def _build():
    nc = bacc.Bacc("TRN2", num_devices=NCORES)
    # x0h packed: [F0, NG, FREE//2] f32 words = bf16 pairs (PACKED) or
    # plain bf16 [F0, NG, FREE].
    if PACKED:
        x0h = nc.declare_dram_parameter("x0h", [F0, NG, FREE // 2], f32,
                                        isOutput=False)
    else:
        x0h = nc.declare_dram_parameter("x0h", [F0, NG, FREE], bf16,
                                        isOutput=False)
    xh = nc.declare_dram_parameter("xh", [F, NG, FREE], bf16, isOutput=False)
    kch = nc.declare_dram_parameter("kch", [CONTRACT, NCH, M], bf16, isOutput=False)
    pph = nc.declare_dram_parameter("pph", [CONTRACT, NCH, 128],
                                    f32 if PACKED else bf16, isOutput=False)
    outp = nc.declare_dram_parameter("out", [NB, M, D], f32, isOutput=True)

    gp_route = {2, 7, 12} if N_GP == 3 else set(range(N_GP))

    with ExitStack() as ctx:
        tc = ctx.enter_context(tile.TileContext(nc))
        singles = ctx.enter_context(tc.tile_pool(name="singles", bufs=1))
        xr_pool = ctx.enter_context(tc.tile_pool(name="xr", bufs=3))
        x0_pool = ctx.enter_context(tc.tile_pool(name="x0", bufs=3))
        sb_pool = ctx.enter_context(tc.tile_pool(name="sb", bufs=4))
        a_pool = ctx.enter_context(tc.tile_pool(name="a", bufs=4))
        osb_pool = ctx.enter_context(tc.tile_pool(name="osb", bufs=3))
        psa_pool = ctx.enter_context(tc.tile_pool(name="psa", bufs=5, space="PSUM"))
        pso_pool = ctx.enter_context(tc.tile_pool(name="pso", bufs=2, space="PSUM"))

        kcw = singles.tile([CONTRACT, NCH, M], bf16)
        nc.sync.dma_start(out=kcw, in_=kch[:, :, :])
        ppw = singles.tile([CONTRACT, NCH, 128], f32 if PACKED else bf16)
        nc.sync.dma_start(out=ppw, in_=pph[:, :, :])

        XW = FREE // 2 if PACKED else FREE

        for g in range(NG):
            bsl = slice(g * GB, (g + 1) * GB)
            xr = xr_pool.tile([CONTRACT, FREE], bf16, tag="xr")
            for r in range(3):
                nc.sync.dma_start(out=xr[r * 40:(r + 1) * 40, :], in_=xh[:, g, :])
            # x0t rows 0-39 real; rows 40-119 copies (finite, zero-weighted)
            x0t = x0_pool.tile([CONTRACT, XW], f32 if PACKED else bf16, tag="x0t")
            for r in range(3):
                nc.sync.dma_start(out=x0t[r * 40:(r + 1) * 40, :], in_=x0h[:, g, :])

            pso = pso_pool.tile([M, FREE], f32, tag="pso")
            acs = [None] * NCH

            def issue_repl(c):
                psa = psa_pool.tile([128, XW], f32, tag="psa")
                nc.tensor.matmul(psa, ppw[:, c, :], x0t, start=True, stop=True)
                ac = a_pool.tile([CONTRACT, FREE], bf16, tag="ac")
                psa_view = (psa[0:CONTRACT, :].bitcast(bf16) if PACKED
                            else psa[0:CONTRACT, :])
                if c in gp_route:
                    sba = sb_pool.tile([CONTRACT, FREE], bf16, tag="sba")
                    nc.scalar.copy(sba, psa_view)
                    nc.gpsimd.tensor_tensor(out=ac, in0=sba, in1=xr,
                                            op=mybir.AluOpType.mult)
                else:
                    nc.vector.tensor_tensor(out=ac, in0=psa_view, in1=xr,
                                            op=mybir.AluOpType.mult)
                acs[c] = ac

            def issue_real(c):
                nc.tensor.matmul(pso, kcw[:, c, :], acs[c],
                                 start=(c == 0), stop=(c == NCH - 1))

            # Waves of 5 chunks: PE sees same-dtype runs (repl fp32 runs,
            # contraction bf16 runs) instead of per-MM dtype alternation.
            waves = [list(range(0, 5)), list(range(5, 10)), list(range(10, 14))]
            for w, wave in enumerate(waves):
                for c in wave:
                    issue_repl(c)
                if w >= 1:
                    for c in waves[w - 1]:
                        issue_real(c)
            for c in waves[-1]:
                issue_real(c)

            osb = osb_pool.tile([M, GB, D], f32, tag="osb")
            nc.scalar.activation(osb.rearrange("m b d -> m (b d)"), pso,
                                 mybir.ActivationFunctionType.Sigmoid)
            nc.scalar.dma_start(out=outp[bsl].transpose([1, 0, 2]), in_=osb)

    nc.finalize()
    return nc


_NC_CACHE = {}


def _get_nc():
    if "nc" not in _NC_CACHE:
        _NC_CACHE["nc"] = _build()
    return _NC_CACHE["nc"]


def _in_maps(x0: np.ndarray, x: np.ndarray, kernel_np: np.ndarray):
    kc, pp = _pack_consts(np.asarray(kernel_np, dtype=np.float32))
    x0 = np.asarray(x0, dtype=np.float32).astype(BF16NP)
    x = np.asarray(x, dtype=np.float32).astype(BF16NP)
    maps = []
    for i in range(NCORES):
        sl = slice(i * NB, (i + 1) * NB)
        x0c = np.ascontiguousarray(
            x0[sl].transpose(1, 0, 2).reshape(F0, NG, FREE))
        if PACKED:
            x0c = x0c.view(np.float32)          # [F0, NG, FREE//2] bf16-pairs
        xc = np.ascontiguousarray(
            x[sl].transpose(1, 0, 2).reshape(F, NG, FREE))
        maps.append({"x0h": x0c, "xh": xc, "kch": kc, "pph": pp})
    return maps


def kernel(x0: np.ndarray, x: np.ndarray, kernel: np.ndarray) -> np.ndarray:
    nc = _get_nc()
    in_maps = _in_maps(x0, x, kernel)
    res = run_bass_kernel_spmd(nc, in_maps, list(range(NCORES)))
    out = np.concatenate([np.asarray(r["out"]) for r in res.results], axis=0)
    return out.astype(np.float32)


# revision 6
# speedup vs baseline: 5.3077x; 1.0388x over previous
"""Trainium2 Bass kernel for CIN: out[b,m,d] = sigmoid(einsum('bid,bjd,ijm', x0, x, K)).

v6: pair-group packed replication.

The x0 partition-broadcast runs as a 0/1 replication matmul in fp32 over
host-packed bf16 PAIRS. fp32 matmuls lower to TWO hardware passes, so one
N=512 fp32 replication matmul covering TWO groups' packed x0 costs the
same as two N=256 ones covering one group each -- but produces PSUM that
the DVE drains at the packed-bitcast 2x rate for both groups at once
(600 ns per chunk-pair vs 2x690 ns unpacked).

Per group-pair (16 b's, 2 x 512 free):
  - repl MM c: psa2[128, 512] f32 = pp_c.T @ x0t2 (packed pairs, 2 groups)
  - DVE: ac2[120, 2, 512] bf16 = bitcast(psa2) * xr2   (one op, 2x mode)
  - 2 x 14 bf16 K=120 contraction MMs accumulate into pso0/pso1
  - ACT sigmoid evacuation x2, DMA out x2
All PE matmuls are K=120 (uniform row-config; mixing K serializes the PE).
Issue order batches same-dtype runs (fp32 repl waves vs bf16 contraction
waves) to avoid per-instruction dtype alternation.
"""

import sys

for _p in ("/opt/trn_rl_repo", "/root/.axon_site/_ro/trn_rl_repo"):
    if _p not in sys.path:
        sys.path.insert(0, _p)

from contextlib import ExitStack

import numpy as np
import ml_dtypes

import concourse.bass as bass
from concourse import bacc
import concourse.tile as tile
from concourse import mybir
from concourse.bass_utils import run_bass_kernel_spmd

B, F0, F, D, M = 4096, 40, 40, 64, 128
NCORES = 8
NB = B // NCORES            # 512
GB = 8
FREE = GB * D               # 512
NG = NB // GB               # 64 groups, processed in 32 pairs
NCH = 14
CONTRACT = 120

f32 = mybir.dt.float32
bf16 = mybir.dt.bfloat16
BF16NP = ml_dtypes.bfloat16

REPL_F32R = False           # flip if float32r passthrough verifies


def _pack_consts(kernel_np: np.ndarray):
    kc = np.zeros((CONTRACT, NCH, M), np.float32)
    pp = np.zeros((CONTRACT, NCH, 128), np.float32)
    for c in range(NCH):
        for t in range(3):
            i = 3 * c + t
            if i < F0:
                kc[t * 40:(t + 1) * 40, c, :] = kernel_np[i]
                pp[i, c, t * 40:(t + 1) * 40] = 1.0
    return kc.astype(BF16NP), pp


def _build():
    nc = bacc.Bacc("TRN2", num_devices=NCORES)
    # x0 packed pairs: [F0, NG, FREE//2] f32 words
    x0h = nc.declare_dram_parameter("x0h", [F0, NG, FREE // 2], f32,
                                    isOutput=False)
    xh = nc.declare_dram_parameter("xh", [F, NG, FREE], bf16, isOutput=False)
    kch = nc.declare_dram_parameter("kch", [CONTRACT, NCH, M], bf16, isOutput=False)
    pph = nc.declare_dram_parameter("pph", [CONTRACT, NCH, 128], f32, isOutput=False)
    outp = nc.declare_dram_parameter("out", [NB, M, D], f32, isOutput=True)

    XW = FREE // 2          # packed words per group

    with ExitStack() as ctx:
        tc = ctx.enter_context(tile.TileContext(nc))
        singles = ctx.enter_context(tc.tile_pool(name="singles", bufs=1))
        xr_pool = ctx.enter_context(tc.tile_pool(name="xr", bufs=3))
        x0_pool = ctx.enter_context(tc.tile_pool(name="x0", bufs=3))
        a_pool = ctx.enter_context(tc.tile_pool(name="a", bufs=4))
        osb_pool = ctx.enter_context(tc.tile_pool(name="osb", bufs=4))
        psa_pool = ctx.enter_context(tc.tile_pool(name="psa", bufs=4, space="PSUM"))
        pso_pool = ctx.enter_context(tc.tile_pool(name="pso", bufs=2, space="PSUM"))

        kcw = singles.tile([CONTRACT, NCH, M], bf16)
        nc.sync.dma_start(out=kcw, in_=kch[:, :, :])
        ppw = singles.tile([CONTRACT, NCH, 128], f32)
        nc.sync.dma_start(out=ppw, in_=pph[:, :, :])
        ppv = ppw.bitcast(mybir.dt.float32r) if REPL_F32R else ppw

        for gp in range(NG // 2):
            g0, g1 = 2 * gp, 2 * gp + 1

            xr2 = xr_pool.tile([CONTRACT, 2, FREE], bf16, tag="xr2")
            for h, g in enumerate((g0, g1)):
                for r in range(3):
                    nc.sync.dma_start(out=xr2[r * 40:(r + 1) * 40, h, :],
                                      in_=xh[:, g, :])
            # packed x0 for both groups; rows 40-119 finite copies
            x0t2 = x0_pool.tile([CONTRACT, 2, XW], f32, tag="x0t2")
            for h, g in enumerate((g0, g1)):
                for r in range(3):
                    nc.sync.dma_start(out=x0t2[r * 40:(r + 1) * 40, h, :],
                                      in_=x0h[:, g, :])
            x0t2f = x0t2.rearrange("p g w -> p (g w)")
            x0t2v = x0t2f.bitcast(mybir.dt.float32r) if REPL_F32R else x0t2f

            pso0 = pso_pool.tile([M, FREE], f32, tag="pso0")
            pso1 = pso_pool.tile([M, FREE], f32, tag="pso1")
            psos = (pso0, pso1)
            acs = [None] * NCH

            def issue_repl(c):
                psa2 = psa_pool.tile([128, 2 * XW], f32, tag="psa2")
                nc.tensor.matmul(psa2, ppv[:, c, :], x0t2v, start=True, stop=True)
                ac2 = a_pool.tile([CONTRACT, 2, FREE], bf16, tag="ac2")
                nc.vector.tensor_tensor(
                    out=ac2,
                    in0=psa2[0:CONTRACT, :].bitcast(bf16)
                        .rearrange("p (g f) -> p g f", g=2),
                    in1=xr2, op=mybir.AluOpType.mult)
                acs[c] = ac2

            def issue_real(c):
                for h in range(2):
                    nc.tensor.matmul(psos[h], kcw[:, c, :], acs[c][:, h, :],
                                     start=(c == 0), stop=(c == NCH - 1))

            waves = [list(range(0, 5)), list(range(5, 10)), list(range(10, 14))]
            for w, wave in enumerate(waves):
                for c in wave:
                    issue_repl(c)
                if w >= 1:
                    for c in waves[w - 1]:
                        issue_real(c)
            for c in waves[-1]:
                issue_real(c)

            for h, g in enumerate((g0, g1)):
                bsl = slice(g * GB, (g + 1) * GB)
                osb = osb_pool.tile([M, GB, D], f32, tag="osb")
                nc.scalar.activation(osb.rearrange("m b d -> m (b d)"), psos[h],
                                     mybir.ActivationFunctionType.Sigmoid)
                nc.scalar.dma_start(out=outp[bsl].transpose([1, 0, 2]), in_=osb)

    nc.finalize()
    return nc


_NC_CACHE = {}


def _get_nc():
    if "nc" not in _NC_CACHE:
        _NC_CACHE["nc"] = _build()
    return _NC_CACHE["nc"]


def _in_maps(x0: np.ndarray, x: np.ndarray, kernel_np: np.ndarray):
    kc, pp = _pack_consts(np.asarray(kernel_np, dtype=np.float32))
    x0 = np.asarray(x0, dtype=np.float32).astype(BF16NP)
    x = np.asarray(x, dtype=np.float32).astype(BF16NP)
    maps = []
    for i in range(NCORES):
        sl = slice(i * NB, (i + 1) * NB)
        x0c = np.ascontiguousarray(
            x0[sl].transpose(1, 0, 2).reshape(F0, NG, FREE)).view(np.float32)
        xc = np.ascontiguousarray(
            x[sl].transpose(1, 0, 2).reshape(F, NG, FREE))
        maps.append({"x0h": x0c, "xh": xc, "kch": kc, "pph": pp})
    return maps


def kernel(x0: np.ndarray, x: np.ndarray, kernel: np.ndarray) -> np.ndarray:
    nc = _get_nc()
    in_maps = _in_maps(x0, x, kernel)
    res = run_bass_kernel_spmd(nc, in_maps, list(range(NCORES)))
    out = np.concatenate([np.asarray(r["out"]) for r in res.results], axis=0)
    return out.astype(np.float32)


# revision 7
# speedup vs baseline: 6.0828x; 1.1460x over previous
"""Trainium2 Bass kernel for CIN: out[b,m,d] = sigmoid(einsum('bid,bjd,ijm', x0, x, K)).

v7: uniform-K bf16 replication, engine-balanced evacuation.

Microbenchmark-derived rules baked in:
  - PE matmuls stream at ~216 ns/MM (N=512 warm) only when every MM has
    the SAME contraction row-config and dtype; K-mixing serializes to
    ~540 ns, fp32 moving operands pump at half rate AND lower to 2 HW
    passes. So: every matmul is bf16 with K=120.
  - PSUM-sourced elementwise ops run at 1x: DVE tensor_tensor 690 ns,
    ACT copy 720 ns per [*,512] chunk. The per-group broadcast evac
    (14 chunks) is split: 8 chunks DVE-direct, 5 chunks ACT-copy +
    GPSIMD-multiply, 1 chunk ACT-copy + DVE 2x multiply.

Per group (8 b's, free=512): 14 bf16 K=120 replication MMs (psa =
pp_c.T @ x0t broadcasts x0 rows across partitions), per-chunk multiply
ac = psa * xr on DVE/GP, 14 bf16 K=120 contraction MMs accumulate, ACT
sigmoid evacuation, DMA out. PE: 28 MMs/group = the pacer (~6 us).
"""

import sys

for _p in ("/opt/trn_rl_repo", "/root/.axon_site/_ro/trn_rl_repo"):
    if _p not in sys.path:
        sys.path.insert(0, _p)

from contextlib import ExitStack

import numpy as np
import ml_dtypes

import concourse.bass as bass
from concourse import bacc
import concourse.tile as tile
from concourse import mybir
from concourse.bass_utils import run_bass_kernel_spmd

B, F0, F, D, M = 4096, 40, 40, 64, 128
NCORES = 8
NB = B // NCORES            # 512
GB = 8
FREE = GB * D               # 512
NG = NB // GB               # 64
NCH = 14
CONTRACT = 120

f32 = mybir.dt.float32
bf16 = mybir.dt.bfloat16
BF16NP = ml_dtypes.bfloat16

GP_ROUTE = {2, 5, 8, 11, 13}    # ACT-copy + GPSIMD multiply
ACTDVE_ROUTE = {0}              # ACT-copy + DVE 2x multiply
LOOKAHEAD = 2


def _pack_consts(kernel_np: np.ndarray):
    kc = np.zeros((CONTRACT, NCH, M), np.float32)
    pp = np.zeros((CONTRACT, NCH, 128), np.float32)
    for c in range(NCH):
        for t in range(3):
            i = 3 * c + t
            if i < F0:
                kc[t * 40:(t + 1) * 40, c, :] = kernel_np[i]
                pp[i, c, t * 40:(t + 1) * 40] = 1.0
    return kc.astype(BF16NP), pp.astype(BF16NP)


def _build():
    nc = bacc.Bacc("TRN2", num_devices=NCORES)
    x0h = nc.declare_dram_parameter("x0h", [F0, NG, FREE], bf16, isOutput=False)
    xh = nc.declare_dram_parameter("xh", [F, NG, FREE], bf16, isOutput=False)
    kch = nc.declare_dram_parameter("kch", [CONTRACT, NCH, M], bf16, isOutput=False)
    pph = nc.declare_dram_parameter("pph", [CONTRACT, NCH, 128], bf16, isOutput=False)
    outp = nc.declare_dram_parameter("out", [NB, M, D], f32, isOutput=True)

    with ExitStack() as ctx:
        tc = ctx.enter_context(tile.TileContext(nc))
        singles = ctx.enter_context(tc.tile_pool(name="singles", bufs=1))
        xr_pool = ctx.enter_context(tc.tile_pool(name="xr", bufs=3))
        x0_pool = ctx.enter_context(tc.tile_pool(name="x0", bufs=3))
        sb_pool = ctx.enter_context(tc.tile_pool(name="sb", bufs=4))
        a_pool = ctx.enter_context(tc.tile_pool(name="a", bufs=4))
        osb_pool = ctx.enter_context(tc.tile_pool(name="osb", bufs=3))
        psa_pool = ctx.enter_context(tc.tile_pool(name="psa", bufs=4, space="PSUM"))
        pso_pool = ctx.enter_context(tc.tile_pool(name="pso", bufs=2, space="PSUM"))

        kcw = singles.tile([CONTRACT, NCH, M], bf16)
        nc.sync.dma_start(out=kcw, in_=kch[:, :, :])
        ppw = singles.tile([CONTRACT, NCH, 128], bf16)
        nc.sync.dma_start(out=ppw, in_=pph[:, :, :])

        for g in range(NG):
            bsl = slice(g * GB, (g + 1) * GB)
            xr = xr_pool.tile([CONTRACT, FREE], bf16, tag="xr")
            for r in range(3):
                nc.sync.dma_start(out=xr[r * 40:(r + 1) * 40, :], in_=xh[:, g, :])
            # x0t rows 0-39 real; rows 40-119 finite copies (zero-weighted)
            x0t = x0_pool.tile([CONTRACT, FREE], bf16, tag="x0t")
            for r in range(3):
                nc.sync.dma_start(out=x0t[r * 40:(r + 1) * 40, :], in_=x0h[:, g, :])

            pso = pso_pool.tile([M, FREE], f32, tag="pso")
            acs = [None] * NCH
            for c in range(NCH + LOOKAHEAD):
                if c < NCH:
                    psa = psa_pool.tile([128, FREE], f32, tag="psa")
                    nc.tensor.matmul(psa, ppw[:, c, :], x0t, start=True, stop=True)
                    ac = a_pool.tile([CONTRACT, FREE], bf16, tag="ac")
                    if c in GP_ROUTE or c in ACTDVE_ROUTE:
                        sba = sb_pool.tile([CONTRACT, FREE], bf16, tag="sba")
                        nc.scalar.copy(sba, psa[0:CONTRACT, :])
                        eng = nc.gpsimd if c in GP_ROUTE else nc.vector
                        eng.tensor_tensor(out=ac, in0=sba, in1=xr,
                                          op=mybir.AluOpType.mult)
                    else:
                        nc.vector.tensor_tensor(out=ac, in0=psa[0:CONTRACT, :],
                                                in1=xr, op=mybir.AluOpType.mult)
                    acs[c] = ac
                if c >= LOOKAHEAD:
                    cc = c - LOOKAHEAD
                    nc.tensor.matmul(pso, kcw[:, cc, :], acs[cc],
                                     start=(cc == 0), stop=(cc == NCH - 1))

            osb = osb_pool.tile([M, GB, D], f32, tag="osb")
            nc.scalar.activation(osb.rearrange("m b d -> m (b d)"), pso,
                                 mybir.ActivationFunctionType.Sigmoid)
            nc.scalar.dma_start(out=outp[bsl].transpose([1, 0, 2]), in_=osb)

    nc.finalize()
    return nc


_NC_CACHE = {}


def _get_nc():
    if "nc" not in _NC_CACHE:
        _NC_CACHE["nc"] = _build()
    return _NC_CACHE["nc"]


def _in_maps(x0: np.ndarray, x: np.ndarray, kernel_np: np.ndarray):
    kc, pp = _pack_consts(np.asarray(kernel_np, dtype=np.float32))
    x0 = np.asarray(x0, dtype=np.float32).astype(BF16NP)
    x = np.asarray(x, dtype=np.float32).astype(BF16NP)
    maps = []
    for i in range(NCORES):
        sl = slice(i * NB, (i + 1) * NB)
        x0c = np.ascontiguousarray(
            x0[sl].transpose(1, 0, 2).reshape(F0, NG, FREE))
        xc = np.ascontiguousarray(
            x[sl].transpose(1, 0, 2).reshape(F, NG, FREE))
        maps.append({"x0h": x0c, "xh": xc, "kch": kc, "pph": pp})
    return maps


def kernel(x0: np.ndarray, x: np.ndarray, kernel: np.ndarray) -> np.ndarray:
    nc = _get_nc()
    in_maps = _in_maps(x0, x, kernel)
    res = run_bass_kernel_spmd(nc, in_maps, list(range(NCORES)))
    out = np.concatenate([np.asarray(r["out"]) for r in res.results], axis=0)
    return out.astype(np.float32)


# revision 8
# speedup vs baseline: 6.2040x; 1.0199x over previous
"""Trainium2 Bass kernel for CIN: out[b,m,d] = sigmoid(einsum('bid,bjd,ijm', x0, x, K)).

v8: uniform-K bf16 replication, engine-balanced evacuation.

Microbenchmark-derived rules baked in:
  - PE matmuls stream at ~216 ns/MM (N=512 warm) only when every MM has
    the SAME contraction row-config and dtype; K-mixing serializes to
    ~540 ns, fp32 moving operands pump at half rate AND lower to 2 HW
    passes. So: every matmul is bf16 with K=120.
  - PSUM-sourced elementwise ops run at 1x: DVE tensor_tensor 690 ns,
    ACT copy 720 ns per [*,512] chunk. The per-group broadcast evac
    (14 chunks) is split: 8 chunks DVE-direct, 5 chunks ACT-copy +
    GPSIMD-multiply, 1 chunk ACT-copy + DVE 2x multiply.

Per group (8 b's, free=512): 14 bf16 K=120 replication MMs (psa =
pp_c.T @ x0t broadcasts x0 rows across partitions), per-chunk multiply
ac = psa * xr on DVE/GP, 14 bf16 K=120 contraction MMs accumulate, ACT
sigmoid evacuation, DMA out. PE: 28 MMs/group = the pacer (~6 us).
"""

import sys

for _p in ("/opt/trn_rl_repo", "/root/.axon_site/_ro/trn_rl_repo"):
    if _p not in sys.path:
        sys.path.insert(0, _p)

from contextlib import ExitStack

import numpy as np
import ml_dtypes

import concourse.bass as bass
from concourse import bacc
import concourse.tile as tile
from concourse import mybir
from concourse.bass_utils import run_bass_kernel_spmd

B, F0, F, D, M = 4096, 40, 40, 64, 128
NCORES = 8
NB = B // NCORES            # 512
GB = 8
FREE = GB * D               # 512
NG = NB // GB               # 64
NCH = 14
CONTRACT = 120

f32 = mybir.dt.float32
bf16 = mybir.dt.bfloat16
BF16NP = ml_dtypes.bfloat16

GP_ROUTE = {2, 5, 8, 11, 13}    # ACT-copy + GPSIMD multiply
ACTDVE_ROUTE = {4}              # ACT-copy + DVE 2x multiply
LOOKAHEAD = 3


def _pack_consts(kernel_np: np.ndarray):
    kc = np.zeros((CONTRACT, NCH, M), np.float32)
    pp = np.zeros((CONTRACT, NCH, 128), np.float32)
    for c in range(NCH):
        for t in range(3):
            i = 3 * c + t
            if i < F0:
                kc[t * 40:(t + 1) * 40, c, :] = kernel_np[i]
                pp[i, c, t * 40:(t + 1) * 40] = 1.0
    return kc.astype(BF16NP), pp.astype(BF16NP)


def _build():
    nc = bacc.Bacc("TRN2", num_devices=NCORES)
    x0h = nc.declare_dram_parameter("x0h", [F0, NG, FREE], bf16, isOutput=False)
    xh = nc.declare_dram_parameter("xh", [F, NG, FREE], bf16, isOutput=False)
    kch = nc.declare_dram_parameter("kch", [CONTRACT, NCH, M], bf16, isOutput=False)
    pph = nc.declare_dram_parameter("pph", [CONTRACT, NCH, 128], bf16, isOutput=False)
    outp = nc.declare_dram_parameter("out", [NB, M, D], f32, isOutput=True)

    with ExitStack() as ctx:
        tc = ctx.enter_context(tile.TileContext(nc))
        singles = ctx.enter_context(tc.tile_pool(name="singles", bufs=1))
        xr_pool = ctx.enter_context(tc.tile_pool(name="xr", bufs=3))
        x0_pool = ctx.enter_context(tc.tile_pool(name="x0", bufs=3))
        sb_pool = ctx.enter_context(tc.tile_pool(name="sb", bufs=6))
        a_pool = ctx.enter_context(tc.tile_pool(name="a", bufs=6))
        osb_pool = ctx.enter_context(tc.tile_pool(name="osb", bufs=3))
        psa_pool = ctx.enter_context(tc.tile_pool(name="psa", bufs=5, space="PSUM"))
        pso_pool = ctx.enter_context(tc.tile_pool(name="pso", bufs=3, space="PSUM"))

        kcw = singles.tile([CONTRACT, NCH, M], bf16)
        nc.sync.dma_start(out=kcw, in_=kch[:, :, :])
        ppw = singles.tile([CONTRACT, NCH, 128], bf16)
        nc.sync.dma_start(out=ppw, in_=pph[:, :, :])

        for g in range(NG):
            bsl = slice(g * GB, (g + 1) * GB)
            xr = xr_pool.tile([CONTRACT, FREE], bf16, tag="xr")
            for r in range(3):
                nc.sync.dma_start(out=xr[r * 40:(r + 1) * 40, :], in_=xh[:, g, :])
            # x0t rows 0-39 real; rows 40-119 finite copies (zero-weighted)
            x0t = x0_pool.tile([CONTRACT, FREE], bf16, tag="x0t")
            for r in range(3):
                nc.sync.dma_start(out=x0t[r * 40:(r + 1) * 40, :], in_=x0h[:, g, :])

            pso = pso_pool.tile([M, FREE], f32, tag="pso")
            acs = [None] * NCH
            for c in range(NCH + LOOKAHEAD):
                if c < NCH:
                    psa = psa_pool.tile([128, FREE], f32, tag="psa")
                    nc.tensor.matmul(psa, ppw[:, c, :], x0t, start=True, stop=True)
                    ac = a_pool.tile([CONTRACT, FREE], bf16, tag="ac")
                    if c in GP_ROUTE or c in ACTDVE_ROUTE:
                        sba = sb_pool.tile([CONTRACT, FREE], bf16, tag="sba")
                        nc.scalar.copy(sba, psa[0:CONTRACT, :])
                        eng = nc.gpsimd if c in GP_ROUTE else nc.vector
                        eng.tensor_tensor(out=ac, in0=sba, in1=xr,
                                          op=mybir.AluOpType.mult)
                    else:
                        nc.vector.tensor_tensor(out=ac, in0=psa[0:CONTRACT, :],
                                                in1=xr, op=mybir.AluOpType.mult)
                    acs[c] = ac
                if c >= LOOKAHEAD:
                    cc = c - LOOKAHEAD
                    nc.tensor.matmul(pso, kcw[:, cc, :], acs[cc],
                                     start=(cc == 0), stop=(cc == NCH - 1))

            osb = osb_pool.tile([M, GB, D], f32, tag="osb")
            nc.scalar.activation(osb.rearrange("m b d -> m (b d)"), pso,
                                 mybir.ActivationFunctionType.Sigmoid)
            nc.scalar.dma_start(out=outp[bsl].transpose([1, 0, 2]), in_=osb)

    nc.finalize()
    return nc


_NC_CACHE = {}


def _get_nc():
    if "nc" not in _NC_CACHE:
        _NC_CACHE["nc"] = _build()
    return _NC_CACHE["nc"]


def _in_maps(x0: np.ndarray, x: np.ndarray, kernel_np: np.ndarray):
    kc, pp = _pack_consts(np.asarray(kernel_np, dtype=np.float32))
    x0 = np.asarray(x0, dtype=np.float32).astype(BF16NP)
    x = np.asarray(x, dtype=np.float32).astype(BF16NP)
    maps = []
    for i in range(NCORES):
        sl = slice(i * NB, (i + 1) * NB)
        x0c = np.ascontiguousarray(
            x0[sl].transpose(1, 0, 2).reshape(F0, NG, FREE))
        xc = np.ascontiguousarray(
            x[sl].transpose(1, 0, 2).reshape(F, NG, FREE))
        maps.append({"x0h": x0c, "xh": xc, "kch": kc, "pph": pp})
    return maps


def kernel(x0: np.ndarray, x: np.ndarray, kernel: np.ndarray) -> np.ndarray:
    nc = _get_nc()
    in_maps = _in_maps(x0, x, kernel)
    res = run_bass_kernel_spmd(nc, in_maps, list(range(NCORES)))
    out = np.concatenate([np.asarray(r["out"]) for r in res.results], axis=0)
    return out.astype(np.float32)
